# revision 2
# baseline (speedup 1.0000x reference)
"""Trainium2 Bass kernel for nn_Attention_7962869366891.

Module: y = x + Wo @ attn(LN_q(x) Wq, LN_c(x) Wkv)   with B=4, N=2048, F=1024,
H=16 heads, D=64.

Sharding (8 cores): core c -> (batch b = c//2, head-group g = c%2 of 8 heads).
Each core computes a full [N, F] partial of the output projection for its 8
heads; the host sums the two partials per batch plus the residual skip.

Device-side design (per core), v2 (overlap-restructured, bf16 datapath):
  - x arrives feature-major bf16 (xT [F, N], host transpose + cast).
  - LN stats via PE: S1 = ones^T x, S2 = ones^T x^2, broadcast across
    partitions for free.  rstd via Ln/Exp on ACT.
  - LN affine folded into weights on host; the per-token (-mu*r) correction
    rides as ONE K=2 matmul per accumulation group ([-rowsum(W'); bias] x
    [mu*r; 1]).
  - All matmuls bf16 operands, fp32 PSUM accumulate.
  - Attention per head: St[m,n] = k^T q (key-major), exp on ACT straight out
    of PSUM into bf16 pt, O^T = V'^T P with a ones column on V giving the
    softmax denominator.  No max-subtraction: logits ~N(0,1).
  - Issue order streams head-pairs: Q/K of pair p+1 are issued after pair
    p's attention so the Tile scheduler uses them as PE filler while the
    ScalarE exp stream (the ~300us floor) runs continuously.
  - Normalize via reciprocal + gpsimd partition_broadcast (DRAM bounce).
  - Output projection contracts heads; skip + pair-sum on host; out bf16.
"""

import numpy as np
import ml_dtypes

import concourse.bass as bass
import concourse.bacc as bacc
import concourse.mybir as mybir
import concourse.tile as tile
from concourse.bass_utils import run_bass_kernel_spmd

F32 = mybir.dt.float32
BF16 = mybir.dt.bfloat16
AF = mybir.ActivationFunctionType

B, N, F, H, D = 4, 2048, 1024, 16, 64
HG = 8                # heads per core
E = HG * D            # 512 projection dims per core
NT = N // 128         # 16 token tiles
FT = F // 128         # 8 feature tiles
ET = E // 128         # 4 e-tiles (head pairs)
NCH = N // 512        # 4 token chunks of 512
EPS = 1e-5

_CACHE = {}


def build_program():
    nc = bacc.Bacc("TRN2", target_bir_lowering=False, debug=False, num_devices=8)

    xT = nc.dram_tensor("xT", [F, N], BF16, kind="ExternalInput").ap()
    wq = nc.dram_tensor("wq", [F, E], BF16, kind="ExternalInput").ap()
    wk = nc.dram_tensor("wk", [F, E], BF16, kind="ExternalInput").ap()
    wv = nc.dram_tensor("wv", [F, E], BF16, kind="ExternalInput").ap()
    corr = nc.dram_tensor("corr", [2, 3 * E], BF16, kind="ExternalInput").ap()
    wo = nc.dram_tensor("wo", [E, F], BF16, kind="ExternalInput").ap()
    onesd = nc.dram_tensor("onesd", [128, 512], BF16, kind="ExternalInput").ap()
    out = nc.dram_tensor("out", [N, F], BF16, kind="ExternalOutput").ap()
    scr = nc.dram_tensor("scr", [HG * NCH, 512], F32).ap()

    with tile.TileContext(nc) as tc:
        _emit(nc, tc, xT, wq, wk, wv, corr, wo, onesd, out, scr)
    nc.compile()
    return nc


def _emit(nc, tc, xT, wq, wk, wv, corr, wo, onesd, out, scr):
    from contextlib import ExitStack
    pers = ExitStack()
    with pers:
        # ---------------- persistent constants ----------------
        single = pers.enter_context(tc.tile_pool(name="single", bufs=1))
        ones128 = single.tile([128, 128], BF16)
        nc.sync.dma_start(out=ones128, in_=onesd[:, 0:128])
        zero_c = single.tile([128, 1], F32)
        nc.vector.memset(zero_c, 0.0)
        eps_c = single.tile([128, 1], F32)
        nc.vector.memset(eps_c, EPS)
        aug = single.tile([2, N], BF16)        # row0 = mu*rstd, row1 = ones
        for c in range(NCH):
            nc.sync.dma_start(out=aug[1:2, c * 512:(c + 1) * 512],
                              in_=onesd[0:1, :])
        corr2 = single.tile([2, 3 * E], BF16)  # row0 = -rowsum(W'), row1 = bias
        nc.sync.dma_start(out=corr2, in_=corr)

        # ---------------- weights (all upfront) ----------------
        wpool = pers.enter_context(tc.tile_pool(name="w", bufs=1))
        wq_sb, wk_sb, wv_sb = [], [], []
        for wdram, lst, nm in ((wq, wq_sb, "wq"), (wk, wk_sb, "wk"),
                               (wv, wv_sb, "wv")):
            for ft in range(FT):
                t = wpool.tile([128, E], BF16, name=f"{nm}_{ft}",
                               tag=f"{nm}_{ft}")
                nc.sync.dma_start(out=t, in_=wdram[ft * 128:(ft + 1) * 128, :])
                lst.append(t)
        wo_sb = []
        for et in range(ET):
            t = wpool.tile([128, F], BF16, name=f"wo_{et}", tag=f"wo_{et}")
            nc.sync.dma_start(out=t, in_=wo[et * 128:(et + 1) * 128, :])
            wo_sb.append(t)

        # ---------------- x + LN ----------------
        xpool = pers.enter_context(tc.tile_pool(name="x", bufs=1))
        xt = xpool.tile([128, FT * N], BF16)
        xt_r = xt.rearrange("p (f n) -> p f n", n=N)
        for ft in range(FT):
            nc.sync.dma_start(out=xt_r[:, ft, :],
                              in_=xT[ft * 128:(ft + 1) * 128, :])

        with tc.tile_pool(name="pstats", bufs=1, space="PSUM") as pstats, \
             tc.tile_pool(name="xsq", bufs=2) as xsqp, \
             tc.tile_pool(name="statf", bufs=2) as statf, \
             tc.tile_pool(name="rp", bufs=1) as rp:
            rb = rp.tile([128, N], F32)  # rstd, broadcast across partitions
            s1 = [pstats.tile([128, 512], F32, tag=f"s1{c}", name=f"s1_{c}")
                  for c in range(NCH)]
            s2 = [pstats.tile([128, 512], F32, tag=f"s2{c}", name=f"s2_{c}")
                  for c in range(NCH)]
            for ft in range(FT):
                for c in range(NCH):
                    cs = slice(c * 512, (c + 1) * 512)
                    xs = xsqp.tile([128, 512], BF16, tag="xsq")
                    nc.vector.tensor_mul(xs, xt_r[:, ft, cs], xt_r[:, ft, cs])
                    nc.tensor.matmul(s1[c], ones128, xt_r[:, ft, cs],
                                     start=(ft == 0), stop=(ft == FT - 1))
                    nc.tensor.matmul(s2[c], ones128, xs,
                                     start=(ft == 0), stop=(ft == FT - 1))
            for c in range(NCH):
                cs = slice(c * 512, (c + 1) * 512)
                mu = statf.tile([128, 512], F32, tag="mu")
                ms = statf.tile([128, 512], F32, tag="ms")
                m2 = statf.tile([128, 512], F32, tag="m2")
                nc.vector.tensor_scalar_mul(mu, s1[c], 1.0 / F)
                nc.vector.tensor_scalar_mul(ms, s2[c], 1.0 / F)
                nc.vector.tensor_mul(m2, mu, mu)
                nc.vector.tensor_sub(ms, ms, m2)   # ms = var
                nc.scalar.activation(m2, ms, AF.Ln, bias=eps_c)
                nc.scalar.activation(rb[:, cs], m2, AF.Exp,
                                     bias=zero_c, scale=-0.5)
                nc.vector.tensor_mul(mu, mu, rb[:, cs])          # mu*rstd
                nc.vector.tensor_copy(aug[0:1, cs], mu[0:1, :])  # -> bf16
            # ---- z = x * rstd (in place, bf16) ----
            for ft in range(FT):
                for c in range(NCH):
                    cs = slice(c * 512, (c + 1) * 512)
                    nc.vector.tensor_mul(xt_r[:, ft, cs], xt_r[:, ft, cs],
                                         rb[:, cs])

        # ---------------- per-pair q/k, token-major v ----------------
        qkpool = pers.enter_context(tc.tile_pool(name="qk", bufs=1, side="right"))
        qt = [qkpool.tile([128, N], BF16, name=f"qt_{et}", tag=f"qt_{et}")
              for et in range(ET)]
        kt = [qkpool.tile([128, N], BF16, name=f"kt_{et}", tag=f"kt_{et}")
              for et in range(ET)]
        vpool = pers.enter_context(tc.tile_pool(name="vtok", bufs=1, side="right"))
        vt = [vpool.tile([128, HG * (D + 1)], BF16, name=f"vt_{m}",
                         tag=f"vt_{m}") for m in range(NT)]
        vt_r = [t.rearrange("p (h x) -> p h x", x=D + 1) for t in vt]
        opool = pers.enter_context(tc.tile_pool(name="ostk", bufs=1, side="right"))
        ot = [[opool.tile([128, 512], BF16, name=f"ot_{et}_{c}",
                          tag=f"ot_{et}_{c}") for c in range(NCH)]
              for et in range(ET)]

        def proj_qk(pool, wsb, wi, et, dest):
            crA = corr2[:, wi * E + et * 128: wi * E + (et + 1) * 128]
            for c in range(NCH):
                cs = slice(c * 512, (c + 1) * 512)
                ps = pool.tile([128, 512], F32, tag="pp",
                               name=f"pp{wi}_{et}_{c}")
                for ft in range(FT):
                    nc.tensor.matmul(ps, wsb[ft][:, et * 128:(et + 1) * 128],
                                     xt_r[:, ft, cs],
                                     start=(ft == 0), stop=False)
                nc.tensor.matmul(ps, crA, aug[:, cs], start=False, stop=True)
                nc.vector.tensor_copy(dest[et][:, cs], ps)

        def proj_v(pool):
            vcr = corr2[:, 2 * E:3 * E]
            for m in range(NT):
                ts_ = slice(m * 128, (m + 1) * 128)
                nc.sync.dma_start(out=vt_r[m][:, :, D:D + 1],
                                  in_=onesd[:, 0:HG])
                ps = pool.tile([128, 512], F32, tag="pp", name=f"ppv_{m}")
                for ft in range(FT):
                    nc.tensor.matmul(ps, xt_r[:, ft, ts_], wv_sb[ft],
                                     start=(ft == 0), stop=False)
                nc.tensor.matmul(ps, aug[:, ts_], vcr, start=False, stop=True)
                nc.vector.tensor_copy(
                    vt_r[m][:, :, 0:D],
                    ps.rearrange("p (h d) -> p h d", d=D))

        qkv = ExitStack()
        with qkv:
            with tc.tile_pool(name="vwork", bufs=3, space="PSUM") as vwork:
                proj_qk(vwork, wq_sb, 0, 0, qt)
                proj_qk(vwork, wk_sb, 1, 0, kt)
                proj_v(vwork)

            # ---------------- attention + filler projections ----------------
            with tc.tile_pool(name="pst", bufs=2, space="PSUM") as pst, \
                 tc.tile_pool(name="po", bufs=3, space="PSUM") as po, \
                 tc.tile_pool(name="pwork", bufs=1, space="PSUM") as pwork, \
                 tc.tile_pool(name="ptp", bufs=3) as ptp, \
                 tc.tile_pool(name="dn", bufs=4) as dnp:

                def attention_pair(p):
                    for h in (2 * p, 2 * p + 1):
                        er = (h % 2) * 64
                        for nh in range(2):
                            o_ps = [po.tile([65, 512], F32, tag="ops",
                                            name=f"ops{h}_{nh}_{i}")
                                    for i in range(2)]
                            for m in range(NT):
                                ms_ = slice(m * 128, (m + 1) * 128)
                                st = pst.tile([128, 1024], F32, tag="st",
                                              name=f"st{h}_{nh}_{m}")
                                for i in range(2):
                                    c = 2 * nh + i
                                    cs = slice(c * 512, (c + 1) * 512)
                                    nc.tensor.matmul(
                                        st[:, i * 512:(i + 1) * 512],
                                        kt[p][er:er + 64, ms_],
                                        qt[p][er:er + 64, cs],
                                        start=True, stop=True)
                                pt = ptp.tile([128, 1024], BF16, tag="pt",
                                              name=f"pt{h}_{nh}_{m}")
                                nc.scalar.activation(pt, st, AF.Exp,
                                                     bias=zero_c)
                                for i in range(2):
                                    nc.tensor.matmul(
                                        o_ps[i], vt_r[m][:, h, :],
                                        pt[:, i * 512:(i + 1) * 512],
                                        start=(m == 0), stop=(m == NT - 1))
                            for i in range(2):
                                c = 2 * nh + i
                                rr = dnp.tile([1, 512], F32, tag="rr",
                                              name=f"rr{h}_{c}")
                                nc.vector.reciprocal(rr, o_ps[i][64:65, :])
                                rbt = dnp.tile([64, 512], F32, tag="rb",
                                               name=f"rbt{h}_{c}")
                                sr = scr[h * NCH + c:h * NCH + c + 1, :]
                                nc.gpsimd.dma_start(out=sr, in_=rr)
                                nc.gpsimd.dma_start(out=rbt,
                                                    in_=sr.to_broadcast([64, 512]))
                                nc.vector.tensor_mul(ot[p][c][er:er + 64, :],
                                                     o_ps[i][0:64, :], rbt)

                attention_pair(0)
                proj_qk(pwork, wq_sb, 0, 1, qt)
                proj_qk(pwork, wk_sb, 1, 1, kt)
                attention_pair(1)
                proj_qk(pwork, wq_sb, 0, 2, qt)
                proj_qk(pwork, wk_sb, 1, 2, kt)
                attention_pair(2)
                proj_qk(pwork, wq_sb, 0, 3, qt)
                proj_qk(pwork, wk_sb, 1, 3, kt)
                attention_pair(3)

                # ---------------- output projection ----------------
                with tc.tile_pool(name="obp", bufs=4) as obp:
                    for tt in range(NT):
                        c = tt // 4
                        js = slice((tt % 4) * 128, (tt % 4 + 1) * 128)
                        ts_ = slice(tt * 128, (tt + 1) * 128)
                        for fc in range(2):
                            fs = slice(fc * 512, (fc + 1) * 512)
                            ps = pwork.tile([128, 512], F32, tag="pp",
                                            name=f"pso{tt}_{fc}")
                            for et in range(ET):
                                nc.tensor.matmul(ps, ot[et][c][:, js],
                                                 wo_sb[et][:, fs],
                                                 start=(et == 0),
                                                 stop=(et == ET - 1))
                            ob = obp.tile([128, 512], BF16, tag="ob",
                                          name=f"ob{tt}_{fc}")
                            nc.vector.tensor_copy(ob, ps)
                            nc.sync.dma_start(out=out[ts_, fs], in_=ob)


def _prep(inputs):
    x = np.asarray(inputs["x"], np.float32)
    Wq = np.asarray(inputs["Wq"], np.float32)
    Wkv = np.asarray(inputs["Wkv"], np.float32)
    Wo = np.asarray(inputs["Wo"], np.float32)
    ln_g = np.asarray(inputs["ln_g"], np.float32)
    ln_b = np.asarray(inputs["ln_b"], np.float32)
    lnc_g = np.asarray(inputs["lnc_g"], np.float32)
    lnc_b = np.asarray(inputs["lnc_b"], np.float32)

    bf = ml_dtypes.bfloat16
    qscale = np.float32(D ** -0.5)
    in_maps = []
    for c in range(8):
        b, g = c // 2, c % 2
        gs = slice(g * E, (g + 1) * E)
        Wq_g = Wq[gs] * ln_g[None, :] * qscale          # [E, F] (scale folded)
        cq = (Wq[gs] @ ln_b) * qscale                   # [E]
        Wk_g = Wkv[gs] * lnc_g[None, :]
        ck = Wkv[gs] @ lnc_b
        Wv_g = Wkv[H * D + g * E:H * D + (g + 1) * E] * lnc_g[None, :]
        cv = Wkv[H * D + g * E:H * D + (g + 1) * E] @ lnc_b
        corr = np.stack([
            np.concatenate([-Wq_g.sum(1), -Wk_g.sum(1), -Wv_g.sum(1)]),
            np.concatenate([cq, ck, cv]),
        ])                                              # [2, 3E]
        in_maps.append({
            "onesd": np.ones((128, 512), bf),
            "xT": np.ascontiguousarray(x[b].T).astype(bf),
            "wq": np.ascontiguousarray(Wq_g.T).astype(bf),
            "wk": np.ascontiguousarray(Wk_g.T).astype(bf),
            "wv": np.ascontiguousarray(Wv_g.T).astype(bf),
            "corr": np.ascontiguousarray(corr).astype(bf),
            "wo": np.ascontiguousarray(Wo[:, gs].T).astype(bf),
        })
    return in_maps


def kernel(**inputs):
    if "nc" not in _CACHE:
        _CACHE["nc"] = build_program()
    nc = _CACHE["nc"]
    in_maps = _prep(inputs)
    res = run_bass_kernel_spmd(nc, in_maps, list(range(8))).results
    x = np.asarray(inputs["x"], np.float32)
    out = np.empty((B, N, F), np.float32)
    for b in range(B):
        out[b] = (res[2 * b]["out"].astype(np.float32)
                  + res[2 * b + 1]["out"].astype(np.float32)
                  + x[b])
    return out


if __name__ == "__main__":
    import reference
    ins = {k: np.asarray(v) for k, v in reference.setup_inputs().items()}
    exp = np.asarray(reference.reference(**ins))
    got = kernel(**ins)
    err = np.abs(got - exp)
    rel = np.linalg.norm(got - exp) / np.linalg.norm(exp)
    print("max abs err:", err.max(), "rel:", rel)


# revision 3
# speedup vs baseline: 1.2048x; 1.2048x over previous
"""Trainium2 Bass kernel for nn_Attention_7962869366891.

Module: y = x + Wo @ attn(LN_q(x) Wq, LN_c(x) Wkv)   with B=4, N=2048, F=1024,
H=16 heads, D=64.

Sharding (8 cores): core c -> (batch b = c//2, head-group g = c%2 of 8 heads).
Each core computes a full [N, F] partial of the output projection for its 8
heads; the host sums the two partials per batch plus the residual skip.

Device-side design (per core), v3:
  - bf16 datapath, fp32 PSUM/stats/normalization.
  - x feature-major in 32 [128,512] tiles (per ft x chunk) for fine deps.
  - LN stats via PE (S1 = ones^T x, S2 = ones^T x^2); rstd via batched
    Ln then Exp on ACT (one table set, no thrash).  LN affine folded into
    weights host-side; per-token -mu*rstd rides as one K=2 matmul per
    accumulation group.
  - Attention per (nhalf, head): St[m,n] = k^T q, exp straight out of PSUM
    into bf16 pt, O^T = V'^T P with a ones column on V for the softmax
    denominator.  No max-subtraction: logits ~N(0,1).
  - ScalarE exp (~295us) is the floor; PE must never gap (HAM clock-gate):
    filler PSUM-group closures (V proj, next pair's Q/K, out-proj chunks)
    are emitted INSIDE the attention m-loops at a paced rate so the PE
    queue always has ready work while waiting on exp.
  - o_ps PSUM slots released early via a DVE copy; the reciprocal
    broadcast (gpsimd DRAM bounce) then runs off-critical-path.
"""

import numpy as np
import ml_dtypes

import concourse.bass as bass
import concourse.bacc as bacc
import concourse.mybir as mybir
import concourse.tile as tile
from concourse.bass_utils import run_bass_kernel_spmd

F32 = mybir.dt.float32
BF16 = mybir.dt.bfloat16
AF = mybir.ActivationFunctionType

B, N, F, H, D = 4, 2048, 1024, 16, 64
HG = 8                # heads per core
E = HG * D            # 512 projection dims per core
NT = N // 128         # 16 token tiles
FT = F // 8           # feature tile count = 8 (128 each)
FTC = 8
ET = E // 128         # 4 e-tiles (head pairs)
NCH = N // 512        # 4 token chunks of 512
EPS = 1e-5

_CACHE = {}


def build_program():
    nc = bacc.Bacc("TRN2", target_bir_lowering=False, debug=False, num_devices=8)

    xT = nc.dram_tensor("xT", [F, N], BF16, kind="ExternalInput").ap()
    wq = nc.dram_tensor("wq", [F, E], BF16, kind="ExternalInput").ap()
    wk = nc.dram_tensor("wk", [F, E], BF16, kind="ExternalInput").ap()
    wv = nc.dram_tensor("wv", [F, E], BF16, kind="ExternalInput").ap()
    corr = nc.dram_tensor("corr", [2, 3 * E], BF16, kind="ExternalInput").ap()
    wo = nc.dram_tensor("wo", [E, F], BF16, kind="ExternalInput").ap()
    onesd = nc.dram_tensor("onesd", [128, 512], BF16, kind="ExternalInput").ap()
    out = nc.dram_tensor("out", [N, F], BF16, kind="ExternalOutput").ap()
    scr = nc.dram_tensor("scr", [HG * NCH, 512], F32).ap()

    with tile.TileContext(nc) as tc:
        _emit(nc, tc, xT, wq, wk, wv, corr, wo, onesd, out, scr)
    nc.compile()
    return nc


def _emit(nc, tc, xT, wq, wk, wv, corr, wo, onesd, out, scr):
    from contextlib import ExitStack
    pers = ExitStack()
    with pers:
        # ---------------- persistent constants ----------------
        single = pers.enter_context(tc.tile_pool(name="single", bufs=1))
        ones128 = single.tile([128, 128], BF16)
        nc.sync.dma_start(out=ones128, in_=onesd[:, 0:128])
        zero_c = single.tile([128, 1], F32)
        nc.vector.memset(zero_c, 0.0)
        eps_c = single.tile([128, 1], F32)
        nc.vector.memset(eps_c, EPS)
        aug = single.tile([2, N], BF16)        # row0 = mu*rstd, row1 = ones
        for c in range(NCH):
            nc.sync.dma_start(out=aug[1:2, c * 512:(c + 1) * 512],
                              in_=onesd[0:1, :])
        corr2 = single.tile([2, 3 * E], BF16)  # row0 = -rowsum(W'), row1 = bias
        nc.sync.dma_start(out=corr2, in_=corr)

        # ---------------- x (32 fine tiles) ----------------
        xpool = pers.enter_context(tc.tile_pool(name="x", bufs=1))
        xt = [[xpool.tile([128, 512], BF16, name=f"x_{ft}_{c}",
                          tag=f"x_{ft}_{c}") for c in range(NCH)]
              for ft in range(FTC)]
        for ft in range(FTC):
            for c in range(NCH):
                nc.sync.dma_start(
                    out=xt[ft][c],
                    in_=xT[ft * 128:(ft + 1) * 128, c * 512:(c + 1) * 512])

        # ---------------- weights (all upfront) ----------------
        wpool = pers.enter_context(tc.tile_pool(name="w", bufs=1))
        wq_sb, wk_sb, wv_sb = [], [], []
        for wdram, lst, nm in ((wq, wq_sb, "wq"), (wk, wk_sb, "wk"),
                               (wv, wv_sb, "wv")):
            for ft in range(FTC):
                t = wpool.tile([128, E], BF16, name=f"{nm}_{ft}",
                               tag=f"{nm}_{ft}")
                nc.sync.dma_start(out=t, in_=wdram[ft * 128:(ft + 1) * 128, :])
                lst.append(t)
        wo_sb = []
        for et in range(ET):
            t = wpool.tile([128, F], BF16, name=f"wo_{et}", tag=f"wo_{et}")
            nc.sync.dma_start(out=t, in_=wo[et * 128:(et + 1) * 128, :])
            wo_sb.append(t)

        # ---------------- LN stats ----------------
        rp = pers.enter_context(tc.tile_pool(name="rp", bufs=1))
        rb = [rp.tile([128, 512], F32, name=f"rb_{c}", tag=f"rb_{c}")
              for c in range(NCH)]
        with tc.tile_pool(name="pstats", bufs=1, space="PSUM") as pstats, \
             tc.tile_pool(name="xsq", bufs=2) as xsqp, \
             tc.tile_pool(name="statf", bufs=4) as statf:
            s1 = [pstats.tile([128, 512], F32, tag=f"s1{c}", name=f"s1_{c}")
                  for c in range(NCH)]
            s2 = [pstats.tile([128, 512], F32, tag=f"s2{c}", name=f"s2_{c}")
                  for c in range(NCH)]
            for ft in range(FTC):
                for c in range(NCH):
                    xs = xsqp.tile([128, 512], BF16, tag="xsq")
                    nc.vector.tensor_mul(xs, xt[ft][c], xt[ft][c])
                    nc.tensor.matmul(s1[c], ones128, xt[ft][c],
                                     start=(ft == 0), stop=(ft == FTC - 1))
                    nc.tensor.matmul(s2[c], ones128, xs,
                                     start=(ft == 0), stop=(ft == FTC - 1))
            mus, lns = [], []
            for c in range(NCH):
                mu = statf.tile([128, 512], F32, tag=f"mu{c}", name=f"mu_{c}")
                ms = statf.tile([128, 512], F32, tag=f"ms{c}", name=f"ms_{c}")
                nc.vector.tensor_scalar_mul(mu, s1[c], 1.0 / F)
                nc.vector.tensor_scalar_mul(ms, s2[c], 1.0 / F)
                m2 = statf.tile([128, 512], F32, tag=f"m2{c}", name=f"m2_{c}")
                nc.vector.tensor_mul(m2, mu, mu)
                nc.vector.tensor_sub(ms, ms, m2)   # var
                mus.append(mu)
                lns.append(ms)
            for c in range(NCH):   # batched Ln
                nc.scalar.activation(lns[c], lns[c], AF.Ln, bias=eps_c)
            for c in range(NCH):   # batched Exp
                nc.scalar.activation(rb[c], lns[c], AF.Exp,
                                     bias=zero_c, scale=-0.5)
            for c in range(NCH):
                cs = slice(c * 512, (c + 1) * 512)
                nc.vector.tensor_mul(mus[c], mus[c], rb[c])          # mu*rstd
                nc.vector.tensor_copy(aug[0:1, cs], mus[c][0:1, :])  # -> bf16
            # ---- z = x * rstd (in place, bf16) ----
            for ft in range(FTC):
                for c in range(NCH):
                    nc.vector.tensor_mul(xt[ft][c], xt[ft][c], rb[c])

        # ---------------- result tiles ----------------
        qkpool = pers.enter_context(tc.tile_pool(name="qk", bufs=1, side="right"))
        qt = [qkpool.tile([128, N], BF16, name=f"qt_{et}", tag=f"qt_{et}")
              for et in range(ET)]
        kt = [qkpool.tile([128, N], BF16, name=f"kt_{et}", tag=f"kt_{et}")
              for et in range(ET)]
        vpool = pers.enter_context(tc.tile_pool(name="vtok", bufs=1, side="right"))
        vt = [vpool.tile([128, HG * (D + 1)], BF16, name=f"vt_{m}",
                         tag=f"vt_{m}") for m in range(NT)]
        vt_r = [t.rearrange("p (h x) -> p h x", x=D + 1) for t in vt]
        opool = pers.enter_context(tc.tile_pool(name="ostk", bufs=1, side="right"))
        ot = [[opool.tile([128, 512], BF16, name=f"ot_{et}_{c}",
                          tag=f"ot_{et}_{c}") for c in range(NCH)]
              for et in range(ET)]

        qkv = ExitStack()
        with qkv:
            pst = qkv.enter_context(tc.tile_pool(name="pst", bufs=2, space="PSUM"))
            po = qkv.enter_context(tc.tile_pool(name="po", bufs=2, space="PSUM"))
            pwork = qkv.enter_context(tc.tile_pool(name="pwork", bufs=2,
                                                   space="PSUM"))
            ptp = qkv.enter_context(tc.tile_pool(name="ptp", bufs=3))
            dnp = qkv.enter_context(tc.tile_pool(name="dn", bufs=4))
            oup = qkv.enter_context(tc.tile_pool(name="ou", bufs=4))
            obp = qkv.enter_context(tc.tile_pool(name="obp", bufs=4))

            # -------- filler group closures (each: one pwork round-trip) ----
            def qk_group(wsb, wi, et, c, dest):
                def emit():
                    cs = slice(c * 512, (c + 1) * 512)
                    crA = corr2[:, wi * E + et * 128: wi * E + (et + 1) * 128]
                    ps = pwork.tile([128, 512], F32, tag="pp",
                                    name=f"pp{wi}_{et}_{c}")
                    for ft in range(FTC):
                        nc.tensor.matmul(ps,
                                         wsb[ft][:, et * 128:(et + 1) * 128],
                                         xt[ft][c],
                                         start=(ft == 0), stop=False)
                    nc.tensor.matmul(ps, crA, aug[:, cs],
                                     start=False, stop=True)
                    nc.vector.tensor_copy(dest[et][:, cs], ps)
                return emit

            def v_group(m):
                def emit():
                    c, js = m // 4, slice((m % 4) * 128, (m % 4 + 1) * 128)
                    ts_ = slice(m * 128, (m + 1) * 128)
                    nc.sync.dma_start(out=vt_r[m][:, :, D:D + 1],
                                      in_=onesd[:, 0:HG])
                    ps = pwork.tile([128, 512], F32, tag="pp",
                                    name=f"ppv_{m}")
                    for ft in range(FTC):
                        nc.tensor.matmul(ps, xt[ft][c][:, js], wv_sb[ft],
                                         start=(ft == 0), stop=False)
                    nc.tensor.matmul(ps, aug[:, ts_], corr2[:, 2 * E:3 * E],
                                     start=False, stop=True)
                    nc.vector.tensor_copy(
                        vt_r[m][:, :, 0:D],
                        ps.rearrange("p (h d) -> p h d", d=D))
                return emit

            def outproj_group(tt, fc):
                def emit():
                    c = tt // 4
                    js = slice((tt % 4) * 128, (tt % 4 + 1) * 128)
                    ts_ = slice(tt * 128, (tt + 1) * 128)
                    fs = slice(fc * 512, (fc + 1) * 512)
                    ps = pwork.tile([128, 512], F32, tag="pp",
                                    name=f"pso{tt}_{fc}")
                    for et in range(ET):
                        nc.tensor.matmul(ps, ot[et][c][:, js],
                                         wo_sb[et][:, fs],
                                         start=(et == 0), stop=(et == ET - 1))
                    ob = obp.tile([128, 512], BF16, tag="ob",
                                  name=f"ob{tt}_{fc}")
                    nc.vector.tensor_copy(ob, ps)
                    nc.sync.dma_start(out=out[ts_, fs], in_=ob)
                return emit

            filler = []
            fidx = [0]

            def emit_filler(n=1):
                while n > 0 and fidx[0] < len(filler):
                    filler[fidx[0]]()
                    fidx[0] += 1
                    n -= 1

            def flush_filler():
                emit_filler(len(filler))

            # -------- one attention block: (head, nhalf), 16 m-iters --------
            def attn_block(p, h, nh, pace):
                er = (h % 2) * 64
                o_ps = [po.tile([65, 512], F32, tag="ops",
                                name=f"ops{h}_{nh}_{i}") for i in range(2)]
                for m in range(NT):
                    ms_ = slice(m * 128, (m + 1) * 128)
                    st = pst.tile([128, 1024], F32, tag="st",
                                  name=f"st{h}_{nh}_{m}")
                    for i in range(2):
                        c = 2 * nh + i
                        cs = slice(c * 512, (c + 1) * 512)
                        nc.tensor.matmul(st[:, i * 512:(i + 1) * 512],
                                         kt[p][er:er + 64, ms_],
                                         qt[p][er:er + 64, cs],
                                         start=True, stop=True)
                    pt = ptp.tile([128, 1024], BF16, tag="pt",
                                  name=f"pt{h}_{nh}_{m}")
                    nc.scalar.activation(pt, st, AF.Exp, bias=zero_c)
                    for i in range(2):
                        nc.tensor.matmul(o_ps[i], vt_r[m][:, h, :],
                                         pt[:, i * 512:(i + 1) * 512],
                                         start=(m == 0), stop=(m == NT - 1))
                    if pace and m % pace == pace - 1:
                        emit_filler(1)
                for i in range(2):
                    c = 2 * nh + i
                    # free the PSUM slot fast: copy numerator + denominator
                    rr = dnp.tile([1, 512], F32, tag="rr", name=f"rr{h}_{c}")
                    nc.vector.reciprocal(rr, o_ps[i][64:65, :])
                    ou = oup.tile([64, 512], F32, tag="ou", name=f"ou{h}_{c}")
                    nc.vector.tensor_copy(ou, o_ps[i][0:64, :])
                    # broadcast 1/den across partitions via DRAM bounce
                    rbt = dnp.tile([64, 512], F32, tag="rb", name=f"rbt{h}_{c}")
                    sr = scr[h * NCH + c:h * NCH + c + 1, :]
                    nc.gpsimd.dma_start(out=sr, in_=rr)
                    nc.gpsimd.dma_start(out=rbt, in_=sr.to_broadcast([64, 512]))
                    nc.vector.tensor_mul(ot[p][c][er:er + 64, :], ou, rbt)

            # -------- schedule --------
            # upfront: q/k for pair 0 (dense PE, warms HAM)
            for c in range(NCH):
                qk_group(wq_sb, 0, 0, c, qt)()
            for c in range(NCH):
                qk_group(wk_sb, 1, 0, c, kt)()
            # V rides pair-0 h0 blocks at 1/iter (prefetch skew 4)
            for m in range(4):
                v_group(m)()
            filler += [v_group(m) for m in range(4, NT)]
            filler += [qk_group(wq_sb, 0, 1, c, qt) for c in range(NCH)]
            filler += [qk_group(wk_sb, 1, 1, c, kt) for c in range(NCH)]

            attn_block(0, 0, 0, pace=1)       # consumes the 12 V groups + 4
            attn_block(0, 1, 0, pace=3)
            attn_block(0, 0, 1, pace=3)
            attn_block(0, 1, 1, pace=3)
            flush_filler()
            filler += [qk_group(wq_sb, 0, 2, c, qt) for c in range(NCH)]
            filler += [qk_group(wk_sb, 1, 2, c, kt) for c in range(NCH)]

            attn_block(1, 2, 0, pace=3)
            attn_block(1, 3, 0, pace=3)
            attn_block(1, 2, 1, pace=3)
            attn_block(1, 3, 1, pace=3)
            flush_filler()
            filler += [qk_group(wq_sb, 0, 3, c, qt) for c in range(NCH)]
            filler += [qk_group(wk_sb, 1, 3, c, kt) for c in range(NCH)]

            attn_block(2, 4, 0, pace=3)
            attn_block(2, 5, 0, pace=3)
            attn_block(2, 4, 1, pace=3)
            attn_block(2, 5, 1, pace=3)
            flush_filler()

            # pair 3: nh0 for both heads first so out-proj chunks 0,1
            # unlock halfway through; their groups ride the nh1 blocks.
            attn_block(3, 6, 0, pace=0)
            attn_block(3, 7, 0, pace=0)
            filler += [outproj_group(tt, fc) for tt in range(8)
                       for fc in range(2)]
            attn_block(3, 6, 1, pace=2)
            attn_block(3, 7, 1, pace=2)
            filler += [outproj_group(tt, fc) for tt in range(8, NT)
                       for fc in range(2)]
            flush_filler()


def _prep(inputs):
    x = np.asarray(inputs["x"], np.float32)
    Wq = np.asarray(inputs["Wq"], np.float32)
    Wkv = np.asarray(inputs["Wkv"], np.float32)
    Wo = np.asarray(inputs["Wo"], np.float32)
    ln_g = np.asarray(inputs["ln_g"], np.float32)
    ln_b = np.asarray(inputs["ln_b"], np.float32)
    lnc_g = np.asarray(inputs["lnc_g"], np.float32)
    lnc_b = np.asarray(inputs["lnc_b"], np.float32)

    bf = ml_dtypes.bfloat16
    qscale = np.float32(D ** -0.5)
    in_maps = []
    for c in range(8):
        b, g = c // 2, c % 2
        gs = slice(g * E, (g + 1) * E)
        Wq_g = Wq[gs] * ln_g[None, :] * qscale          # [E, F] (scale folded)
        cq = (Wq[gs] @ ln_b) * qscale                   # [E]
        Wk_g = Wkv[gs] * lnc_g[None, :]
        ck = Wkv[gs] @ lnc_b
        Wv_g = Wkv[H * D + g * E:H * D + (g + 1) * E] * lnc_g[None, :]
        cv = Wkv[H * D + g * E:H * D + (g + 1) * E] @ lnc_b
        corr = np.stack([
            np.concatenate([-Wq_g.sum(1), -Wk_g.sum(1), -Wv_g.sum(1)]),
            np.concatenate([cq, ck, cv]),
        ])                                              # [2, 3E]
        in_maps.append({
            "onesd": np.ones((128, 512), bf),
            "xT": np.ascontiguousarray(x[b].T).astype(bf),
            "wq": np.ascontiguousarray(Wq_g.T).astype(bf),
            "wk": np.ascontiguousarray(Wk_g.T).astype(bf),
            "wv": np.ascontiguousarray(Wv_g.T).astype(bf),
            "corr": np.ascontiguousarray(corr).astype(bf),
            "wo": np.ascontiguousarray(Wo[:, gs].T).astype(bf),
        })
    return in_maps


def kernel(**inputs):
    if "nc" not in _CACHE:
        _CACHE["nc"] = build_program()
    nc = _CACHE["nc"]
    in_maps = _prep(inputs)
    res = run_bass_kernel_spmd(nc, in_maps, list(range(8))).results
    x = np.asarray(inputs["x"], np.float32)
    out = np.empty((B, N, F), np.float32)
    for b in range(B):
        out[b] = (res[2 * b]["out"].astype(np.float32)
                  + res[2 * b + 1]["out"].astype(np.float32)
                  + x[b])
    return out


if __name__ == "__main__":
    import reference
    ins = {k: np.asarray(v) for k, v in reference.setup_inputs().items()}
    exp = np.asarray(reference.reference(**ins))
    got = kernel(**ins)
    err = np.abs(got - exp)
    rel = np.linalg.norm(got - exp) / np.linalg.norm(exp)
    print("max abs err:", err.max(), "rel:", rel)
